# revision 1
# baseline (speedup 1.0000x reference)
"""Single-head attention (B=4, S=2048, D=1024) on 8 Trainium2 NeuronCores.

Sharding: batch x KEY-half. Core c handles batch b=c//2 and key rows
[1024*h : 1024*(h+1)] with h=c%2. Each core receives x[b] rolled so its own
key rows come first; it computes Q for ALL 2048 (rolled) queries, K/V for its
1024 keys, and outputs the UNNORMALIZED partial attention O~ = exp(S)V plus
partial row-sums. The host un-rolls the query order and combines the pair:
O = (O~_0 + O~_1) / (rs_0 + rs_1).  (No softmax max-subtraction is needed:
scaled scores are ~N(0,1), so exp never overflows, and partials add.)

Per-core pipeline (activations kept [feature, token] transposed so the PE
contracts over partitions):
  A:  x [2048,1024] -> x^T via PE transposes (fp32, exact), resident
  B1: Q^T = Wq^T x^T + bq (all 2048 queries) -> spilled to DRAM
  B2: K^T (own 1024 keys) -> resident [e,k]
  B3: V natural [k,e] (own keys) -> resident (bias via rank-1 ones x bv mm)
  C:  per 256-query quarter: S^T[k,q] = K^T.T @ Q^T directly in transposed
      layout -> exp(scale*s) on ACT writes P^T straight to SBUF (no PE
      transposes!) -> row-sums via ones-vector matmuls -> O~ = P^T.T @ V
      -> DMA out (raw), row-sums DMA'd as [1,256] rows.
Matmuls run as float32r (1 cycle/row at N>=256 vs 4 for fp32).
"""

import os
import sys
from contextlib import ExitStack

import numpy as np

if "/opt/trn_rl_repo" not in sys.path:
    sys.path.insert(0, "/opt/trn_rl_repo")

import concourse.bass as bass
import concourse.bacc as bacc
import concourse.tile as tile
from concourse import mybir
from concourse.bass_utils import run_bass_kernel_spmd

P = 128
S = 2048        # full sequence (queries per core)
SK = 1024       # keys per core (own half)
D = 1024        # model dim
F32 = mybir.dt.float32
F32R = mybir.dt.float32r

DC = D // P     # 8 d-chunks (contraction over model dim)
EC = D // P     # 8 e-chunks (output features)
KC = SK // P    # 8 key chunks (own half)
NT = 512        # moving-operand tile (one PSUM bank of fp32)
QT = 256        # query tile for transposed scores

USE_F32R = os.environ.get("BASS_ATTN_F32R", "1") == "1"
MD = F32R if USE_F32R else F32  # dtype of tiles feeding the PE
SCALE = 1.0 / float(np.sqrt(np.float32(D)))


def build_program() -> bass.Bass:
    nc = bacc.Bacc(
        "TRN2", target_bir_lowering=False, debug=False, num_devices=8)

    def _in(name, shape):
        return nc.dram_tensor(name, shape, F32, kind="ExternalInput").ap()

    xT_d = _in("xT", [D, S])
    wq_d = _in("Wq", [D, D])
    bq_d = _in("bq", [D])
    wk_d = _in("Wk", [D, D])
    bk_d = _in("bk", [D])
    wv_d = _in("Wv", [D, D])
    bv_d = _in("bv", [D])
    o_d = nc.dram_tensor("o_raw", [S, D], F32, kind="ExternalOutput").ap()
    rs_d = nc.dram_tensor("rs_raw", [S], F32, kind="ExternalOutput").ap()

    with tile.TileContext(nc) as tc, ExitStack() as ctx:
        const_p = ctx.enter_context(tc.tile_pool(name="const", bufs=1))
        io_p = ctx.enter_context(tc.tile_pool(name="io", bufs=2))
        xt_p = ctx.enter_context(tc.tile_pool(name="xt", bufs=DC))
        kt_p = ctx.enter_context(tc.tile_pool(name="kt", bufs=EC))
        v_p = ctx.enter_context(tc.tile_pool(name="v", bufs=KC))
        wqk_p = ctx.enter_context(tc.tile_pool(name="wqk", bufs=3))
        wv_p = ctx.enter_context(tc.tile_pool(name="wv", bufs=8))
        stg_p = ctx.enter_context(tc.tile_pool(name="stg", bufs=3))
        evac_p = ctx.enter_context(tc.tile_pool(name="evac", bufs=2))
        qtc_p = ctx.enter_context(tc.tile_pool(name="qtc", bufs=9))
        pt_p = ctx.enter_context(tc.tile_pool(name="ptp", bufs=9))
        st_p = ctx.enter_context(tc.tile_pool(name="stat", bufs=2))
        dram_p = ctx.enter_context(tc.tile_pool(name="spill", bufs=1, space="DRAM"))
        psA = ctx.enter_context(tc.tile_pool(name="psA", bufs=2, space="PSUM"))
        psB = ctx.enter_context(tc.tile_pool(name="psB", bufs=3, space="PSUM"))
        psO = ctx.enter_context(tc.tile_pool(name="psO", bufs=3, space="PSUM"))

        # ---- constants -------------------------------------------------
        bqt = const_p.tile([P, EC], F32)  # bq chunked [p, ec]
        nc.sync.dma_start(bqt[:], bq_d[:].rearrange("(c p) -> p c", p=P))
        bkt = const_p.tile([P, EC], F32)
        nc.sync.dma_start(bkt[:], bk_d[:].rearrange("(c p) -> p c", p=P))
        bvr = const_p.tile([1, D], MD)   # bv as a single row
        nc.gpsimd.dma_start(bvr[:], bv_d[:].rearrange("(o d) -> o d", o=1))
        ones_raw = const_p.tile([P, 1], F32)
        nc.vector.memset(ones_raw[:], 1.0)
        ones = const_p.tile([P, 1], MD)  # column of ones: lhsT for row-sums
        nc.vector.tensor_copy(ones[:], ones_raw[:])
        onesr_raw = const_p.tile([1, P], F32)
        nc.vector.memset(onesr_raw[:], 1.0)
        ones_row = const_p.tile([1, P], MD)  # row of ones: V-bias rank-1 mm
        nc.vector.tensor_copy(ones_row[:], onesr_raw[:])

        qt_spill = dram_p.tile([EC, P, S], MD)

        # first two Wq column blocks before the bulk x stream, so B1 can
        # start the moment x^T lands (gpsimd queue is FIFO)
        wq_early = {}
        for ec in range(2):
            wq = wqk_p.tile([P, DC, P], MD, name="wqc", tag="wqk")
            nc.gpsimd.dma_start(
                wq[:], wq_d[:, ec * P:(ec + 1) * P].rearrange(
                    "(c p) e -> p c e", p=P))
            wq_early[ec] = wq

        # ---- Phase A: x^T loads (host pre-transposed) ------------------
        xt = [xt_p.tile([P, S], MD, name=f"xt{dc}", tag="xt")
              for dc in range(DC)]
        for dc in range(DC):
            if dc % 2 == 0:
                # SWDGE casting DMA: f32 DRAM -> f32r SBUF (rounds on the fly)
                nc.gpsimd.dma_start(xt[dc][:], xT_d[dc * P:(dc + 1) * P, :])
            else:
                # parallel path: HWDGE f32 load + DVE rounding copy
                for sh in range(2):
                    xs = stg_p.tile([P, S // 2], F32, name="xs", tag="xstg")
                    nc.sync.dma_start(
                        xs[:], xT_d[dc * P:(dc + 1) * P,
                                    sh * (S // 2):(sh + 1) * (S // 2)])
                    nc.vector.tensor_copy(
                        xt[dc][:, sh * (S // 2):(sh + 1) * (S // 2)], xs[:])

        # ---- Phase B1: Q^T (all queries) -> spill ----------------------
        # weights as column blocks [d, ec*128:(ec+1)*128] -> [128, DC, 128]
        for ec in range(EC):
            if ec in wq_early:
                wq = wq_early[ec]
            else:
                wq = wqk_p.tile([P, DC, P], MD, name="wqc", tag="wqk")
                nc.gpsimd.dma_start(
                    wq[:], wq_d[:, ec * P:(ec + 1) * P].rearrange(
                        "(c p) e -> p c e", p=P))
            for qt_i in range(S // NT):
                ps = psB.tile([P, NT], F32)
                for dc in range(DC):
                    nc.tensor.matmul(
                        ps[:],
                        (wq[:, dc, :]),
                        (xt[dc][:, qt_i * NT:(qt_i + 1) * NT]),
                        start=(dc == 0), stop=(dc == DC - 1),
                    )
                ev = evac_p.tile([P, NT], MD, name="ev", tag="evac")
                nc.scalar.activation(
                    ev[:], ps[:], mybir.ActivationFunctionType.Identity,
                    bias=bqt[:, ec:ec + 1],
                )
                nc.sync.dma_start(
                    qt_spill[ec, :, qt_i * NT:(qt_i + 1) * NT], ev[:])

        # ---- Phase B2: K^T (own keys) resident ------------------------
        kt = [kt_p.tile([P, SK], MD, name=f"kt{ec}", tag="kt")
              for ec in range(EC)]
        for ec in range(EC):
            wk = wqk_p.tile([P, DC, P], MD, name="wkc", tag="wqk")
            nc.gpsimd.dma_start(
                wk[:], wk_d[:, ec * P:(ec + 1) * P].rearrange(
                    "(c p) e -> p c e", p=P))
            for kt_i in range(SK // NT):
                ps = psB.tile([P, NT], F32)
                for dc in range(DC):
                    nc.tensor.matmul(
                        ps[:],
                        (wk[:, dc, :]),
                        (xt[dc][:, kt_i * NT:(kt_i + 1) * NT]),
                        start=(dc == 0), stop=(dc == DC - 1),
                    )
                nc.scalar.activation(
                    kt[ec][:, kt_i * NT:(kt_i + 1) * NT], ps[:],
                    mybir.ActivationFunctionType.Identity,
                    bias=bkt[:, ec:ec + 1],
                )

        # prefetch first query-quarter Q^T reload while B3 runs
        qtc0 = [qtc_p.tile([P, QT], MD, tag="qtc", name=f"qtc{ec}")
                for ec in range(EC)]
        for ec in range(EC):
            nc.sync.dma_start(qtc0[ec][:], qt_spill[ec, :, 0:QT])

        # ---- Phase B3: V natural [k, e] (own keys) resident ------------
        v = [v_p.tile([P, D], MD, name=f"v{kc}", tag="v") for kc in range(KC)]
        for et in range(D // NT):
            wvh = [wv_p.tile([P, NT], MD, name=f"wvh{dc}", tag="wv")
                   for dc in range(DC)]
            for dc in range(DC):
                nc.gpsimd.dma_start(
                    wvh[dc][:],
                    wv_d[dc * P:(dc + 1) * P, et * NT:(et + 1) * NT])
            for kc in range(KC):
                ps = psB.tile([P, NT], F32)
                for dc in range(DC):
                    nc.tensor.matmul(
                        ps[:],
                        (xt[dc][:, kc * P:(kc + 1) * P]),
                        (wvh[dc][:]),
                        start=(dc == 0), stop=False,
                    )
                # rank-1 bias add: ones_row^T @ bv_row
                nc.tensor.matmul(
                    ps[:],
                    (ones_row[0:1, :]),
                    (bvr[0:1, et * NT:(et + 1) * NT]),
                    start=False, stop=True,
                )
                nc.vector.tensor_copy(
                    v[kc][:, et * NT:(et + 1) * NT], ps[:])

        # ---- Phase C: attention, transposed scores ---------------------
        for qq in range(S // QT):
            if qq == 0:
                qtc = qtc0
            else:
                qtc = [qtc_p.tile([P, QT], MD, tag="qtc", name=f"qtc{ec}")
                       for ec in range(EC)]
                for ec in range(EC):
                    nc.sync.dma_start(
                        qtc[ec][:], qt_spill[ec, :, qq * QT:(qq + 1) * QT])

            # S^T[k, q] per key chunk; exp writes P^T straight to SBUF
            ptt = [pt_p.tile([P, QT], MD, tag="ptp", name=f"ptt{kc}")
                   for kc in range(KC)]
            for kc in range(KC):
                ps = psB.tile([P, QT], F32)
                for ec in range(EC):
                    nc.tensor.matmul(
                        ps[:],
                        (kt[ec][:, kc * P:(kc + 1) * P]),
                        (qtc[ec][:]),
                        start=(ec == 0), stop=(ec == EC - 1),
                    )
                nc.scalar.activation(
                    ptt[kc][:], ps[:],
                    mybir.ActivationFunctionType.Exp,
                    scale=SCALE,
                )

            # O~ = P^T.T @ V, per 128-query chunk. Runs right after the
            # scores matmuls: the O accumulation chain paces slower than the
            # ACT exp stream, absorbing its latency (row-sums would stall).
            for qc in range(QT // P):
                o_sb = io_p.tile([P, D], F32, name="osb", tag="io")
                for et in range(D // NT):
                    ps = psO.tile([P, NT], F32, name="pso")
                    for kc in range(KC):
                        nc.tensor.matmul(
                            ps[:],
                            (ptt[kc][:, qc * P:(qc + 1) * P]),
                            (v[kc][:, et * NT:(et + 1) * NT]),
                            start=(kc == 0), stop=(kc == KC - 1),
                        )
                    nc.vector.tensor_copy(
                        o_sb[:, et * NT:(et + 1) * NT], ps[:])
                row0 = qq * QT + qc * P
                nc.sync.dma_start(o_d[row0:row0 + P, :], o_sb[:])

            # partial row-sums: ones^T @ P^T, accumulated over key chunks
            ps_rs = psA.tile([1, QT], F32, name="ps_rs")
            for kc in range(KC):
                nc.tensor.matmul(
                    ps_rs[:],
                    (ones[:, 0:1]),
                    (ptt[kc][:]),
                    start=(kc == 0), stop=(kc == KC - 1),
                )
            rs_sb = st_p.tile([1, QT], F32, name="rs_sb", tag="rs")
            nc.vector.tensor_copy(rs_sb[:], ps_rs[:])
            nc.sync.dma_start(
                rs_d[qq * QT:(qq + 1) * QT].rearrange("(o q) -> o q", o=1),
                rs_sb[:])

    nc.compile()
    return nc


_CACHE: dict = {}


def _get_program() -> bass.Bass:
    if "nc" not in _CACHE:
        _CACHE["nc"] = build_program()
    return _CACHE["nc"]


def kernel(x, Wq, bq, Wk, bk, Wv, bv, _trace=False, _trace_kwargs=None):
    nc = _get_program()
    x = np.asarray(x, dtype=np.float32)
    shared = {
        "Wq": np.ascontiguousarray(np.asarray(Wq, np.float32)),
        "bq": np.ascontiguousarray(np.asarray(bq, np.float32)),
        "Wk": np.ascontiguousarray(np.asarray(Wk, np.float32)),
        "bk": np.ascontiguousarray(np.asarray(bk, np.float32)),
        "Wv": np.ascontiguousarray(np.asarray(Wv, np.float32)),
        "bv": np.ascontiguousarray(np.asarray(bv, np.float32)),
    }
    in_maps = []
    for c in range(8):
        b, h = divmod(c, 2)
        xb = x[b]
        if h:
            xb = np.roll(xb, -SK, axis=0)  # own key half first
        in_maps.append(
            {"xT": np.ascontiguousarray(xb.T), **shared})

    res = run_bass_kernel_spmd(
        nc, in_maps, list(range(8)),
        trace=_trace, **(_trace_kwargs or {}),
    )
    out = np.empty((4, S, D), dtype=np.float32)
    for b in range(4):
        o0 = res.results[2 * b]["o_raw"].astype(np.float64)
        r0 = res.results[2 * b]["rs_raw"].astype(np.float64)
        o1 = res.results[2 * b + 1]["o_raw"].astype(np.float64)
        r1 = res.results[2 * b + 1]["rs_raw"].astype(np.float64)
        # core h=1 computed queries in rolled order; un-roll before combining
        o1 = np.roll(o1, SK, axis=0)
        r1 = np.roll(r1, SK)
        out[b] = ((o0 + o1) / (r0 + r1)[:, None]).astype(np.float32)
    if _trace:
        return out, res
    return out



# revision 3
# speedup vs baseline: 1.1735x; 1.1735x over previous
"""Single-head attention (B=4, S=2048, D=1024) on 8 Trainium2 NeuronCores.

Sharding: batch x KEY-half. Core c handles batch b=c//2 and key rows
[1024*h : 1024*(h+1)] with h=c%2. Each core receives x[b] rolled so its own
key rows come first; it computes Q for ALL 2048 (rolled) queries, K/V for its
1024 keys, and outputs the UNNORMALIZED partial attention O~ = exp(S)V plus
partial row-sums. The host un-rolls the query order and combines the pair:
O = (O~_0 + O~_1) / (rs_0 + rs_1).  (No softmax max-subtraction is needed:
scaled scores are ~N(0,1), so exp never overflows, and partials add.)

Per-core pipeline (activations kept [feature, token] transposed so the PE
contracts over partitions). Phase order K -> V -> Q -> attention so Q stays
RESIDENT in SBUF (bf16, 4MB) instead of spilling to DRAM:
  x^T arrives in 512-token slices (evens: SWDGE casting DMA, odds: HWDGE +
  DVE rounding copy) so the first K-projection matmul starts ~6us in.
  B2: K^T = Wk^T x^T + bk for own 1024 keys -> resident bf16 [e,k]
  B3: V natural [k,e] -> resident bf16; bias added by DVE with a
      broadcasted bv row (built once by a rank-1 ones x bv matmul)
  B1: Q^T all 2048 queries -> resident bf16 [e,q]
  C:  per 512-query block: S^T[k,q] = K^T.T @ Q^T in transposed layout ->
      exp(scale*s) on ACT writes P^T (bf16) straight to SBUF -> O~ = P^T.T V
      per 128-query chunk -> DMA out raw; row-sums via a DVE add-tree over
      the 8 P^T tiles + ONE ones-vector matmul per block.
Projection matmuls run f32r (1 col/cycle); attention matmuls run bf16.
"""

import sys
from contextlib import ExitStack

import numpy as np

if "/opt/trn_rl_repo" not in sys.path:
    sys.path.insert(0, "/opt/trn_rl_repo")

import concourse.bass as bass
import concourse.bacc as bacc
import concourse.tile as tile
from concourse import mybir
from concourse.bass_utils import run_bass_kernel_spmd

P = 128
S = 2048        # full sequence (queries per core)
SK = 1024       # keys per core (own half)
D = 1024        # model dim
F32 = mybir.dt.float32
F32R = mybir.dt.float32r
BF16 = mybir.dt.bfloat16

DC = D // P     # 8 d-chunks (contraction over model dim)
EC = D // P     # 8 e-chunks (output features)
KC = SK // P    # 8 key chunks (own half)
NT = 512        # moving-operand tile (one PSUM bank of fp32)
QT = 512        # query tile for transposed scores
NSL = S // NT   # 4 token slices of x

SCALE = 1.0 / float(np.sqrt(np.float32(D)))


def build_program() -> bass.Bass:
    nc = bacc.Bacc(
        "TRN2", target_bir_lowering=False, debug=False, num_devices=8)

    def _in(name, shape):
        return nc.dram_tensor(name, shape, F32, kind="ExternalInput").ap()

    xT_d = _in("xT", [D, S])
    wq_d = _in("Wq", [D, D])
    bq_d = _in("bq", [D])
    wk_d = _in("Wk", [D, D])
    bk_d = _in("bk", [D])
    wv_d = _in("Wv", [D, D])
    bv_d = _in("bv", [D])
    o_d = nc.dram_tensor("o_raw", [S, D], F32, kind="ExternalOutput").ap()
    rs_d = nc.dram_tensor("rs_raw", [S], F32, kind="ExternalOutput").ap()

    with tile.TileContext(nc) as tc, ExitStack() as ctx:
        const_p = ctx.enter_context(tc.tile_pool(name="const", bufs=1))
        io_p = ctx.enter_context(tc.tile_pool(name="io", bufs=2))
        xt_p = ctx.enter_context(tc.tile_pool(name="xt", bufs=DC))
        kt_p = ctx.enter_context(tc.tile_pool(name="kt", bufs=EC))
        v_p = ctx.enter_context(tc.tile_pool(name="v", bufs=KC))
        q_p = ctx.enter_context(tc.tile_pool(name="q", bufs=EC))
        wqk_p = ctx.enter_context(tc.tile_pool(name="wqk", bufs=3))
        wv_p = ctx.enter_context(tc.tile_pool(name="wv", bufs=10))
        stg_p = ctx.enter_context(tc.tile_pool(name="stg", bufs=2))
        pt_p = ctx.enter_context(tc.tile_pool(name="ptp", bufs=10))
        rsum_p = ctx.enter_context(tc.tile_pool(name="rsum", bufs=5))
        st_p = ctx.enter_context(tc.tile_pool(name="stat", bufs=1))
        psB = ctx.enter_context(tc.tile_pool(name="psB", bufs=3, space="PSUM"))
        psO = ctx.enter_context(tc.tile_pool(name="psO", bufs=3, space="PSUM"))
        psA = ctx.enter_context(tc.tile_pool(name="psA", bufs=2, space="PSUM"))

        # ---- constants (sync queue: tiny, before x-odd slices) ---------
        bqt = const_p.tile([P, EC], F32)  # bq chunked [p, ec]
        nc.sync.dma_start(bqt[:], bq_d[:].rearrange("(c p) -> p c", p=P))
        bkt = const_p.tile([P, EC], F32)
        nc.sync.dma_start(bkt[:], bk_d[:].rearrange("(c p) -> p c", p=P))
        ones_raw = const_p.tile([P, 1], F32)
        nc.vector.memset(ones_raw[:], 1.0)
        ones = const_p.tile([P, 1], F32R)  # column of ones: lhsT for row-sums
        nc.vector.tensor_copy(ones[:], ones_raw[:])
        onesr_raw = const_p.tile([1, P], F32)
        nc.vector.memset(onesr_raw[:], 1.0)
        ones_row = const_p.tile([1, P], F32R)  # row of ones: bv broadcast mm
        nc.vector.tensor_copy(ones_row[:], onesr_raw[:])

        # ---- DMA prefetch: gpsimd (SWDGE casting) queue ----------------
        # order = arrival order. wk0,wk1 first so B2 can start the moment
        # x slice 0 lands; x even-d slices by token-slice; wk tail blocks
        # are issued inside the B2 loop (buffer rotation).
        wk_blk = {}
        for ec in range(2):
            wk = wqk_p.tile([P, DC, P], F32R, name="wkc", tag="wqk")
            nc.gpsimd.dma_start(
                wk[:], wk_d[:, ec * P:(ec + 1) * P].rearrange(
                    "(c p) e -> p c e", p=P))
            wk_blk[ec] = wk

        xt = [xt_p.tile([P, S], F32R, name=f"xt{dc}", tag="xt")
              for dc in range(DC)]

        def load_x_slice_evens(sl):
            for dc in range(0, DC, 2):
                # SWDGE casting DMA: f32 DRAM -> f32r SBUF (rounds on the fly)
                nc.gpsimd.dma_start(
                    xt[dc][:, sl * NT:(sl + 1) * NT],
                    xT_d[dc * P:(dc + 1) * P, sl * NT:(sl + 1) * NT])

        def load_x_slice_odds(sl):
            for dc in range(1, DC, 2):
                # parallel path: HWDGE f32 load + DVE rounding copy
                xs = stg_p.tile([P, NT], F32, name="xs", tag="xstg")
                nc.sync.dma_start(
                    xs[:], xT_d[dc * P:(dc + 1) * P, sl * NT:(sl + 1) * NT])
                nc.vector.tensor_copy(
                    xt[dc][:, sl * NT:(sl + 1) * NT], xs[:])

        load_x_slice_evens(0)
        load_x_slice_evens(1)
        bvr = const_p.tile([1, D], F32R)   # bv as a single row
        nc.gpsimd.dma_start(bvr[:], bv_d[:].rearrange("(o d) -> o d", o=1))
        for sl in range(NSL):
            load_x_slice_odds(sl)

        # ---- Phase B2: K^T (own keys) resident bf16 --------------------
        kt = [kt_p.tile([P, SK], BF16, name=f"kt{ec}", tag="kt")
              for ec in range(EC)]
        for ec in range(EC):
            if ec in wk_blk:
                wk = wk_blk[ec]
            else:
                wk = wqk_p.tile([P, DC, P], F32R, name="wkc", tag="wqk")
                nc.gpsimd.dma_start(
                    wk[:], wk_d[:, ec * P:(ec + 1) * P].rearrange(
                        "(c p) e -> p c e", p=P))
            for kt_i in range(SK // NT):
                ps = psB.tile([P, NT], F32)
                for dc in range(DC):
                    nc.tensor.matmul(
                        ps[:],
                        (wk[:, dc, :]),
                        (xt[dc][:, kt_i * NT:(kt_i + 1) * NT]),
                        start=(dc == 0), stop=(dc == DC - 1),
                    )
                nc.scalar.activation(
                    kt[ec][:, kt_i * NT:(kt_i + 1) * NT], ps[:],
                    mybir.ActivationFunctionType.Identity,
                    bias=bkt[:, ec:ec + 1],
                )
            # queue the wk blocks two ahead (rotation has 3 buffers)
            if ec + 2 < EC and (ec + 2) not in wk_blk:
                nxt = wqk_p.tile([P, DC, P], F32R, name="wkc", tag="wqk")
                nc.gpsimd.dma_start(
                    nxt[:], wk_d[:, (ec + 2) * P:(ec + 3) * P].rearrange(
                        "(c p) e -> p c e", p=P))
                wk_blk[ec + 2] = nxt

        # wv prefetch right behind the wk tail, before x slices 2/3
        wvh = {}
        for et in range(D // NT):
            for dc in range(DC):
                w = wv_p.tile([P, NT], F32R, name="wvh", tag="wv")
                nc.gpsimd.dma_start(
                    w[:], wv_d[dc * P:(dc + 1) * P, et * NT:(et + 1) * NT])
                wvh[(et, dc)] = w
        load_x_slice_evens(2)
        load_x_slice_evens(3)

        # bv broadcast row -> [128, D] f32 (rank-1 ones x bv matmul)
        bv_rep = const_p.tile([P, D], F32)
        for et in range(D // NT):
            ps = psB.tile([P, NT], F32)
            nc.tensor.matmul(
                ps[:],
                (ones_row[0:1, :]),
                (bvr[0:1, et * NT:(et + 1) * NT]),
                start=True, stop=True,
            )
            nc.scalar.copy(bv_rep[:, et * NT:(et + 1) * NT], ps[:])

        # ---- Phase B3: V natural [k, e] (own keys) resident bf16 -------
        v = [v_p.tile([P, D], BF16, name=f"v{kc}", tag="v")
             for kc in range(KC)]
        for et in range(D // NT):
            for kc in range(KC):
                ps = psB.tile([P, NT], F32)
                for dc in range(DC):
                    nc.tensor.matmul(
                        ps[:],
                        (xt[dc][:, kc * P:(kc + 1) * P]),
                        (wvh[(et, dc)][:]),
                        start=(dc == 0), stop=(dc == DC - 1),
                    )
                nc.vector.tensor_add(
                    v[kc][:, et * NT:(et + 1) * NT], ps[:],
                    bv_rep[:, et * NT:(et + 1) * NT])

        # ---- Phase B1: Q^T (all queries) resident bf16 -----------------
        qres = [q_p.tile([P, S], BF16, name=f"q{ec}", tag="q")
                for ec in range(EC)]
        wq_blk = {}
        wq0 = wqk_p.tile([P, DC, P], F32R, name="wqc", tag="wqk")
        nc.gpsimd.dma_start(
            wq0[:], wq_d[:, 0:P].rearrange("(c p) e -> p c e", p=P))
        wq_blk[0] = wq0
        for ec in range(EC):
            wq = wq_blk[ec]
            if ec + 1 < EC:
                nxt = wqk_p.tile([P, DC, P], F32R, name="wqc", tag="wqk")
                nc.gpsimd.dma_start(
                    nxt[:], wq_d[:, (ec + 1) * P:(ec + 2) * P].rearrange(
                        "(c p) e -> p c e", p=P))
                wq_blk[ec + 1] = nxt
            for qt_i in range(S // NT):
                ps = psB.tile([P, NT], F32)
                for dc in range(DC):
                    nc.tensor.matmul(
                        ps[:],
                        (wq[:, dc, :]),
                        (xt[dc][:, qt_i * NT:(qt_i + 1) * NT]),
                        start=(dc == 0), stop=(dc == DC - 1),
                    )
                nc.scalar.activation(
                    qres[ec][:, qt_i * NT:(qt_i + 1) * NT], ps[:],
                    mybir.ActivationFunctionType.Identity,
                    bias=bqt[:, ec:ec + 1],
                )

        # ---- Phase C: attention, transposed scores ---------------------
        for qq in range(S // QT):
            # S^T[k, q] per key chunk; exp writes P^T (bf16) straight to SBUF
            ptt = [pt_p.tile([P, QT], BF16, tag="ptp", name=f"ptt{kc}")
                   for kc in range(KC)]
            for kc in range(KC):
                ps = psB.tile([P, QT], F32)
                for ec in range(EC):
                    nc.tensor.matmul(
                        ps[:],
                        (kt[ec][:, kc * P:(kc + 1) * P]),
                        (qres[ec][:, qq * QT:(qq + 1) * QT]),
                        start=(ec == 0), stop=(ec == EC - 1),
                    )
                nc.scalar.activation(
                    ptt[kc][:], ps[:],
                    mybir.ActivationFunctionType.Exp,
                    scale=SCALE,
                )

            # O~ = P^T.T @ V, per 128-query chunk. Paces slower than the
            # ACT exp stream, absorbing its latency.
            for qc in range(QT // P):
                o_sb = io_p.tile([P, D], F32, name="osb", tag="io")
                for et in range(D // NT):
                    ps = psO.tile([P, NT], F32, name="pso")
                    for kc in range(KC):
                        nc.tensor.matmul(
                            ps[:],
                            (ptt[kc][:, qc * P:(qc + 1) * P]),
                            (v[kc][:, et * NT:(et + 1) * NT]),
                            start=(kc == 0), stop=(kc == KC - 1),
                        )
                    nc.vector.tensor_copy(
                        o_sb[:, et * NT:(et + 1) * NT], ps[:])
                row0 = qq * QT + qc * P
                nc.sync.dma_start(o_d[row0:row0 + P, :], o_sb[:])

            # partial row-sums: DVE add-tree over the 8 P^T tiles, then a
            # single ones-vector matmul (f32r, 1 col/cycle).
            def _radd(a, b):
                t = rsum_p.tile([P, QT], F32R, name="racc", tag="racc")
                nc.vector.tensor_add(t[:], a, b)
                return t
            s01 = _radd(ptt[0][:], ptt[1][:])
            s23 = _radd(ptt[2][:], ptt[3][:])
            s45 = _radd(ptt[4][:], ptt[5][:])
            s67 = _radd(ptt[6][:], ptt[7][:])
            s03 = _radd(s01[:], s23[:])
            s47 = _radd(s45[:], s67[:])
            tot = _radd(s03[:], s47[:])
            ps_rs = psA.tile([1, QT], F32, name="ps_rs")
            nc.tensor.matmul(
                ps_rs[:], (ones[:, 0:1]), (tot[:]), start=True, stop=True)
            rs_sb = st_p.tile([1, QT], F32, name="rs_sb", tag="rs")
            nc.vector.tensor_copy(rs_sb[:], ps_rs[:])
            nc.sync.dma_start(
                rs_d[qq * QT:(qq + 1) * QT].rearrange("(o q) -> o q", o=1),
                rs_sb[:])

    nc.compile()
    return nc


_CACHE: dict = {}


def _get_program() -> bass.Bass:
    if "nc" not in _CACHE:
        _CACHE["nc"] = build_program()
    return _CACHE["nc"]


def kernel(x, Wq, bq, Wk, bk, Wv, bv, _trace=False, _trace_kwargs=None):
    nc = _get_program()
    x = np.asarray(x, dtype=np.float32)
    shared = {
        "Wq": np.ascontiguousarray(np.asarray(Wq, np.float32)),
        "bq": np.ascontiguousarray(np.asarray(bq, np.float32)),
        "Wk": np.ascontiguousarray(np.asarray(Wk, np.float32)),
        "bk": np.ascontiguousarray(np.asarray(bk, np.float32)),
        "Wv": np.ascontiguousarray(np.asarray(Wv, np.float32)),
        "bv": np.ascontiguousarray(np.asarray(bv, np.float32)),
    }
    in_maps = []
    for c in range(8):
        b, h = divmod(c, 2)
        xb = x[b]
        if h:
            xb = np.roll(xb, -SK, axis=0)  # own key half first
        in_maps.append(
            {"xT": np.ascontiguousarray(xb.T), **shared})

    res = run_bass_kernel_spmd(
        nc, in_maps, list(range(8)),
        trace=_trace, **(_trace_kwargs or {}),
    )
    out = np.empty((4, S, D), dtype=np.float32)
    for b in range(4):
        o0 = res.results[2 * b]["o_raw"].astype(np.float64)
        r0 = res.results[2 * b]["rs_raw"].astype(np.float64)
        o1 = res.results[2 * b + 1]["o_raw"].astype(np.float64)
        r1 = res.results[2 * b + 1]["rs_raw"].astype(np.float64)
        # core h=1 computed queries in rolled order; un-roll before combining
        o1 = np.roll(o1, SK, axis=0)
        r1 = np.roll(r1, SK)
        out[b] = ((o0 + o1) / (r0 + r1)[:, None]).astype(np.float32)
    if _trace:
        return out, res
    return out
